# revision 1
# baseline (speedup 1.0000x reference)
"""AllegroGDML message-passing block on 8 trn2 NeuronCores.

Strategy (per sharding hint): partition the 120000 edges across the 8
cores (15000/core, padded to 15360 = 30*512).  All the heavy per-edge
MLPs run on-device with features on the partition axis and edges on the
free axis (512-edge tiles, one PSUM bank per matmul).  The cheap,
index-dependent glue (scatter-sum to 5000 nodes, env_linear, gather,
the channelwise tensor product and the per-edge generated-weight
contractions) runs on host numpy between the three device launches:

  A: x2b -> lat, w0                     (2-body latent + env0 weights)
  B: latent_in0 -> lw0, lat2, w_env1    (layer-0 gen. weights + resnet)
  C: latent_in1 -> lw1, fw              (layer-1 gen. weights + final)
"""

import numpy as np

MUL = 16
LAT = 128
AVG_N = 20.0
P_CUT = 6.0
E_FULL = 120000
N_NODES = 5000
N_CORES = 8
E_CORE = E_FULL // N_CORES        # 15000
TILE = 512
E_PAD = 15360                     # 30 tiles of 512
N_TILES = E_PAD // TILE

_CACHE = {}


# ----------------------------------------------------------------- host math
def _silu(x):
    return x / (1.0 + np.exp(-x))


def _poly_cutoff(u, p=P_CUT):
    f = (1.0 - ((p + 1.0) * (p + 2.0) / 2.0) * u ** p
         + p * (p + 2.0) * u ** (p + 1.0)
         - (p * (p + 1.0) / 2.0) * u ** (p + 2.0))
    return np.where(u < 1.0, f, 0.0).astype(np.float32)


def _weighter(attr, w):
    w = w.reshape(-1, MUL, 2)
    fs = w[:, :, 0] * attr[:, 0:1]
    fv = w[:, :, 1][:, :, None] * attr[:, None, 1:4]
    return fs, fv


def _scatter(vals, idx, n):
    out = np.zeros((n,) + vals.shape[1:], np.float32)
    np.add.at(out, idx, vals)
    return out


# ------------------------------------------------------------- bass builders
def _mm(nc, psum, lhsT_tiles, rhs_tiles, out_shape):
    """Accumulating matmul over split-K: out = sum_i lhsT_i.T @ rhs_i."""
    n = len(lhsT_tiles)
    for i, (lt, rt) in enumerate(zip(lhsT_tiles, rhs_tiles)):
        nc.tensor.matmul(psum, lt, rt, start=(i == 0), stop=(i == n - 1))


def _build_A(EL=E_PAD):
    import concourse.bacc as bacc
    import concourse.tile as tile
    import concourse.mybir as mybir
    f32 = mybir.dt.float32
    SILU = mybir.ActivationFunctionType.Silu

    nc = bacc.Bacc(None, target_bir_lowering=False, debug=False)
    x2bT = nc.declare_dram_parameter("x2bT", [16, EL], f32, isOutput=False)
    cutB = nc.declare_dram_parameter("cutB", [128, EL], f32, isOutput=False)
    w2b1 = nc.declare_dram_parameter("w2b1", [16, 128], f32, isOutput=False)
    w2b2 = nc.declare_dram_parameter("w2b2", [128, 128], f32, isOutput=False)
    e0w1 = nc.declare_dram_parameter("e0w1", [128, 128], f32, isOutput=False)
    e0w2 = nc.declare_dram_parameter("e0w2", [128, 64], f32, isOutput=False)
    latT = nc.declare_dram_parameter("latT", [128, EL], f32, isOutput=True)
    w0T = nc.declare_dram_parameter("w0T", [64, EL], f32, isOutput=True)

    with tile.TileContext(nc) as tc:
        with (
            tc.tile_pool(name="wpool", bufs=1) as wp,
            tc.tile_pool(name="work", bufs=3) as pool,
            tc.tile_pool(name="psum", bufs=2, space="PSUM") as pp,
        ):
            w2b1_s = wp.tile([16, 128], f32, tag="w2b1")
            w2b2_s = wp.tile([128, 128], f32, tag="w2b2")
            e0w1_s = wp.tile([128, 128], f32, tag="e0w1")
            e0w2_s = wp.tile([128, 64], f32, tag="e0w2")
            nc.sync.dma_start(w2b1_s[:], w2b1[:])
            nc.sync.dma_start(w2b2_s[:], w2b2[:])
            nc.sync.dma_start(e0w1_s[:], e0w1[:])
            nc.sync.dma_start(e0w2_s[:], e0w2[:])

            for t in range(EL // TILE):
                sl = slice(t * TILE, (t + 1) * TILE)
                x_t = pool.tile([16, TILE], f32, tag="x")
                c_t = pool.tile([128, TILE], f32, tag="c")
                nc.sync.dma_start(x_t[:], x2bT[:, sl])
                nc.sync.dma_start(c_t[:], cutB[:, sl])

                ps1 = pp.tile([128, TILE], f32, tag="ps1")
                nc.tensor.matmul(ps1[:], w2b1_s[:], x_t[:], start=True, stop=True)
                h = pool.tile([128, TILE], f32, tag="h")
                nc.scalar.activation(h[:], ps1[:], SILU)

                ps2 = pp.tile([128, TILE], f32, tag="ps2")
                nc.tensor.matmul(ps2[:], w2b2_s[:], h[:], start=True, stop=True)
                lat_t = pool.tile([128, TILE], f32, tag="lat")
                nc.vector.tensor_mul(lat_t[:], ps2[:], c_t[:])
                nc.sync.dma_start(latT[:, sl], lat_t[:])

                ps3 = pp.tile([128, TILE], f32, tag="ps3")
                nc.tensor.matmul(ps3[:], e0w1_s[:], lat_t[:], start=True, stop=True)
                h2 = pool.tile([128, TILE], f32, tag="h2")
                nc.scalar.activation(h2[:], ps3[:], SILU)

                ps4 = pp.tile([64, TILE], f32, tag="ps4")
                nc.tensor.matmul(ps4[:], e0w2_s[:], h2[:], start=True, stop=True)
                w0_t = pool.tile([64, TILE], f32, tag="w0")
                nc.vector.tensor_copy(w0_t[:], ps4[:])
                nc.sync.dma_start(w0T[:, sl], w0_t[:])
    nc.compile()
    return nc


def _build_B(EL=E_PAD):
    import concourse.bacc as bacc
    import concourse.tile as tile
    import concourse.mybir as mybir
    f32 = mybir.dt.float32
    SILU = mybir.ActivationFunctionType.Silu
    a = 0.5
    c_old = float(1.0 / np.sqrt(a * a + 1.0))

    nc = bacc.Bacc(None, target_bir_lowering=False, debug=False)
    liT = nc.declare_dram_parameter("liT", [160, EL], f32, isOutput=False)
    latT = nc.declare_dram_parameter("latT", [128, EL], f32, isOutput=False)
    cutB = nc.declare_dram_parameter("cutB", [128, EL], f32, isOutput=False)
    lw_w1 = nc.declare_dram_parameter("lw_w1", [160, 128], f32, isOutput=False)
    lw_w2 = nc.declare_dram_parameter("lw_w2", [128, 1280], f32, isOutput=False)
    lt_w1 = nc.declare_dram_parameter("lt_w1", [160, 128], f32, isOutput=False)
    lt_w2 = nc.declare_dram_parameter("lt_w2", [128, 128], f32, isOutput=False)
    e1w1 = nc.declare_dram_parameter("e1w1", [128, 128], f32, isOutput=False)
    e1w2 = nc.declare_dram_parameter("e1w2", [128, 32], f32, isOutput=False)
    lwT = nc.declare_dram_parameter("lwT", [1280, EL], f32, isOutput=True)
    lat2T = nc.declare_dram_parameter("lat2T", [128, EL], f32, isOutput=True)
    we1T = nc.declare_dram_parameter("we1T", [32, EL], f32, isOutput=True)

    with tile.TileContext(nc) as tc:
        with (
            tc.tile_pool(name="wpool", bufs=1) as wp,
            tc.tile_pool(name="work", bufs=3) as pool,
            tc.tile_pool(name="ps1", bufs=1, space="PSUM") as pp1,
            tc.tile_pool(name="ps2", bufs=2, space="PSUM") as pp2,
        ):
            lww1a = wp.tile([128, 128], f32, tag="lww1a")
            lww1b = wp.tile([32, 128], f32, tag="lww1b")
            lww2 = wp.tile([128, 1280], f32, tag="lww2")
            ltw1a = wp.tile([128, 128], f32, tag="ltw1a")
            ltw1b = wp.tile([32, 128], f32, tag="ltw1b")
            ltw2 = wp.tile([128, 128], f32, tag="ltw2")
            e1w1_s = wp.tile([128, 128], f32, tag="e1w1")
            e1w2_s = wp.tile([128, 32], f32, tag="e1w2")
            nc.sync.dma_start(lww1a[:], lw_w1[0:128, :])
            nc.sync.dma_start(lww1b[:], lw_w1[128:160, :])
            nc.sync.dma_start(lww2[:], lw_w2[:])
            nc.sync.dma_start(ltw1a[:], lt_w1[0:128, :])
            nc.sync.dma_start(ltw1b[:], lt_w1[128:160, :])
            nc.sync.dma_start(ltw2[:], lt_w2[:])
            nc.sync.dma_start(e1w1_s[:], e1w1[:])
            nc.sync.dma_start(e1w2_s[:], e1w2[:])

            for t in range(EL // TILE):
                sl = slice(t * TILE, (t + 1) * TILE)
                li_a = pool.tile([128, TILE], f32, tag="lia")
                li_b = pool.tile([32, TILE], f32, tag="lib")
                c_t = pool.tile([128, TILE], f32, tag="c")
                lat_t = pool.tile([128, TILE], f32, tag="latin")
                nc.sync.dma_start(li_a[:], liT[0:128, sl])
                nc.sync.dma_start(li_b[:], liT[128:160, sl])
                nc.sync.dma_start(c_t[:], cutB[:, sl])
                nc.sync.dma_start(lat_t[:], latT[:, sl])

                # generated two-body weights: lw = silu(li @ w1) @ w2
                ps_h = pp1.tile([128, TILE], f32, tag="psh")
                _mm(nc, ps_h[:], [lww1a[:], lww1b[:]], [li_a[:], li_b[:]], None)
                h0 = pool.tile([128, TILE], f32, tag="h0")
                nc.scalar.activation(h0[:], ps_h[:], SILU)
                for m in range(10):
                    ps_lw = pp2.tile([128, TILE], f32, tag="pslw")
                    nc.tensor.matmul(ps_lw[:], lww2[:, m * 128:(m + 1) * 128],
                                     h0[:], start=True, stop=True)
                    lw_t = pool.tile([128, TILE], f32, tag="lwt")
                    nc.scalar.activation(
                        lw_t[:], ps_lw[:], mybir.ActivationFunctionType.Copy)
                    nc.sync.dma_start(lwT[m * 128:(m + 1) * 128, sl], lw_t[:])

                # latent resnet: lat2 = c_old*lat + a*c_old*cut*mlp(li)
                ps_n = pp1.tile([128, TILE], f32, tag="psn")
                _mm(nc, ps_n[:], [ltw1a[:], ltw1b[:]], [li_a[:], li_b[:]], None)
                h1 = pool.tile([128, TILE], f32, tag="h1")
                nc.scalar.activation(h1[:], ps_n[:], SILU)
                ps_n2 = pp1.tile([128, TILE], f32, tag="psn2")
                nc.tensor.matmul(ps_n2[:], ltw2[:], h1[:], start=True, stop=True)
                nl = pool.tile([128, TILE], f32, tag="nl")
                nc.vector.tensor_mul(nl[:], ps_n2[:], c_t[:])
                nc.vector.tensor_scalar_mul(nl[:], nl[:], a * c_old)
                l2 = pool.tile([128, TILE], f32, tag="l2")
                nc.vector.tensor_scalar_mul(l2[:], lat_t[:], c_old)
                nc.vector.tensor_add(l2[:], l2[:], nl[:])
                nc.sync.dma_start(lat2T[:, sl], l2[:])

                # w_env1 = silu(lat2 @ e1w1) @ e1w2
                ps_e = pp1.tile([128, TILE], f32, tag="pse")
                nc.tensor.matmul(ps_e[:], e1w1_s[:], l2[:], start=True, stop=True)
                h2 = pool.tile([128, TILE], f32, tag="h2")
                nc.scalar.activation(h2[:], ps_e[:], SILU)
                ps_e2 = pp1.tile([32, TILE], f32, tag="pse2")
                nc.tensor.matmul(ps_e2[:], e1w2_s[:], h2[:], start=True, stop=True)
                we_t = pool.tile([32, TILE], f32, tag="we")
                nc.vector.tensor_copy(we_t[:], ps_e2[:])
                nc.sync.dma_start(we1T[:, sl], we_t[:])
    nc.compile()
    return nc


def _build_C(EL=E_PAD):
    import concourse.bacc as bacc
    import concourse.tile as tile
    import concourse.mybir as mybir
    f32 = mybir.dt.float32
    SILU = mybir.ActivationFunctionType.Silu

    nc = bacc.Bacc(None, target_bir_lowering=False, debug=False)
    liT = nc.declare_dram_parameter("liT", [160, EL], f32, isOutput=False)
    lw_w1 = nc.declare_dram_parameter("lw_w1", [160, 128], f32, isOutput=False)
    lw_w2 = nc.declare_dram_parameter("lw_w2", [128, 1280], f32, isOutput=False)
    f_w1 = nc.declare_dram_parameter("f_w1", [160, 128], f32, isOutput=False)
    f_w2 = nc.declare_dram_parameter("f_w2", [128, 16], f32, isOutput=False)
    lwT = nc.declare_dram_parameter("lwT", [1280, EL], f32, isOutput=True)
    fwT = nc.declare_dram_parameter("fwT", [16, EL], f32, isOutput=True)

    with tile.TileContext(nc) as tc:
        with (
            tc.tile_pool(name="wpool", bufs=1) as wp,
            tc.tile_pool(name="work", bufs=3) as pool,
            tc.tile_pool(name="ps1", bufs=1, space="PSUM") as pp1,
            tc.tile_pool(name="ps2", bufs=2, space="PSUM") as pp2,
        ):
            lww1a = wp.tile([128, 128], f32, tag="lww1a")
            lww1b = wp.tile([32, 128], f32, tag="lww1b")
            lww2 = wp.tile([128, 1280], f32, tag="lww2")
            fw1a = wp.tile([128, 128], f32, tag="fw1a")
            fw1b = wp.tile([32, 128], f32, tag="fw1b")
            fw2 = wp.tile([128, 16], f32, tag="fw2")
            nc.sync.dma_start(lww1a[:], lw_w1[0:128, :])
            nc.sync.dma_start(lww1b[:], lw_w1[128:160, :])
            nc.sync.dma_start(lww2[:], lw_w2[:])
            nc.sync.dma_start(fw1a[:], f_w1[0:128, :])
            nc.sync.dma_start(fw1b[:], f_w1[128:160, :])
            nc.sync.dma_start(fw2[:], f_w2[:])

            for t in range(EL // TILE):
                sl = slice(t * TILE, (t + 1) * TILE)
                li_a = pool.tile([128, TILE], f32, tag="lia")
                li_b = pool.tile([32, TILE], f32, tag="lib")
                nc.sync.dma_start(li_a[:], liT[0:128, sl])
                nc.sync.dma_start(li_b[:], liT[128:160, sl])

                ps_h = pp1.tile([128, TILE], f32, tag="psh")
                _mm(nc, ps_h[:], [lww1a[:], lww1b[:]], [li_a[:], li_b[:]], None)
                h0 = pool.tile([128, TILE], f32, tag="h0")
                nc.scalar.activation(h0[:], ps_h[:], SILU)
                for m in range(10):
                    ps_lw = pp2.tile([128, TILE], f32, tag="pslw")
                    nc.tensor.matmul(ps_lw[:], lww2[:, m * 128:(m + 1) * 128],
                                     h0[:], start=True, stop=True)
                    lw_t = pool.tile([128, TILE], f32, tag="lwt")
                    nc.scalar.activation(
                        lw_t[:], ps_lw[:], mybir.ActivationFunctionType.Copy)
                    nc.sync.dma_start(lwT[m * 128:(m + 1) * 128, sl], lw_t[:])

                ps_f = pp1.tile([128, TILE], f32, tag="psf")
                _mm(nc, ps_f[:], [fw1a[:], fw1b[:]], [li_a[:], li_b[:]], None)
                hf = pool.tile([128, TILE], f32, tag="hf")
                nc.scalar.activation(hf[:], ps_f[:], SILU)
                ps_f2 = pp1.tile([16, TILE], f32, tag="psf2")
                nc.tensor.matmul(ps_f2[:], fw2[:], hf[:], start=True, stop=True)
                fw_t = pool.tile([16, TILE], f32, tag="fwt")
                nc.vector.tensor_copy(fw_t[:], ps_f2[:])
                nc.sync.dma_start(fwT[:, sl], fw_t[:])
    nc.compile()
    return nc


# ------------------------------------------------------------------ run glue
def _shard_T(full_ET):
    """[E, D] host array -> per-core [D, E_PAD] transposed shards."""
    D = full_ET.shape[1]
    shards = []
    for c in range(N_CORES):
        s = np.zeros((D, E_PAD), np.float32)
        s[:, :E_CORE] = full_ET[c * E_CORE:(c + 1) * E_CORE].T
        shards.append(s)
    return shards


def _unshard_T(outs, key, D):
    """per-core [D, E_PAD] -> [E, D] host array."""
    full = np.empty((E_FULL, D), np.float32)
    for c in range(N_CORES):
        full[c * E_CORE:(c + 1) * E_CORE] = outs[c][key][:, :E_CORE].T
    return full


LAST_RESULTS = []


def _run(nc, in_maps):
    import time
    from concourse.bass_utils import run_bass_kernel_spmd
    t0 = time.time()
    r = run_bass_kernel_spmd(nc, in_maps, list(range(N_CORES)))
    LAST_RESULTS.append((r, time.time() - t0))
    return r.results


def kernel(edge_attr, node_attrs, edge_embed, edge_u, edge_index,
           w2b1, w2b2, lat1_w1, lat1_w2, env0_w1, env0_w2, env1_w1, env1_w2,
           l2w0_w1, l2w0_w2, l2w1_w1, l2w1_w2, envlin_ws, envlin_wv,
           fl2w_w1, fl2w_w2):
    LAST_RESULTS.clear()
    E, N = E_FULL, N_NODES
    center, neigh = edge_index[0], edge_index[1]
    cut = _poly_cutoff(np.asarray(edge_u, np.float32))
    norm = np.float32(1.0 / np.sqrt(AVG_N))
    inv_s3 = np.float32(1.0 / np.sqrt(3.0))
    inv_s2 = np.float32(1.0 / np.sqrt(2.0))
    inv_sm = np.float32(1.0 / np.sqrt(MUL))

    x2b = np.concatenate([node_attrs[center], node_attrs[neigh],
                          edge_embed], axis=-1).astype(np.float32)

    cut_shards = _shard_T(cut[:, None])          # [1, E_PAD] per core
    cutB = [np.broadcast_to(s, (128, E_PAD)).copy() for s in cut_shards]

    # ---- launch A
    if "A" not in _CACHE:
        _CACHE["A"] = _build_A()
    in_maps = [{"x2bT": xs, "cutB": cb,
                "w2b1": w2b1, "w2b2": w2b2, "e0w1": env0_w1, "e0w2": env0_w2}
               for xs, cb in zip(_shard_T(x2b), cutB)]
    outs = _run(_CACHE["A"], in_maps)
    lat = _unshard_T(outs, "latT", 128)
    w0 = _unshard_T(outs, "w0T", 64)
    w_edge, w_env = w0[:, :2 * MUL], w0[:, 2 * MUL:]
    fs, fv = _weighter(edge_attr, w_edge)

    def env_block(w_env_l, l):
        es_e, ev_e = _weighter(edge_attr, w_env_l)
        env_s = _scatter(es_e, center, N) * norm
        env_v = _scatter(ev_e, center, N) * norm
        env_s = (env_s @ envlin_ws[l]) * inv_sm
        env_v = np.einsum('nui,uv->nvi', env_v, envlin_wv[l]) * inv_sm
        return env_s[center], env_v[center]

    def tp(fs, fv, es, ev):
        s1 = fs * es
        s2 = np.sum(fv * ev, axis=-1) * inv_s3
        v1 = fs[:, :, None] * ev
        v2 = fv * es[:, :, None]
        v3 = np.cross(fv, ev) * inv_s2
        return s1, s2, v1, v2, v3

    # ---- layer 0 host glue
    es, ev = env_block(w_env, 0)
    s1, s2, v1, v2, v3 = tp(fs, fv, es, ev)
    scalars = np.stack([s1, s2], axis=-1).reshape(E, 2 * MUL)
    latent_in = np.concatenate([lat, scalars], axis=-1)

    # ---- launch B
    if "B" not in _CACHE:
        _CACHE["B"] = _build_B()
    in_maps = [{"liT": ls, "latT": lt, "cutB": cb,
                "lw_w1": l2w0_w1, "lw_w2": l2w0_w2,
                "lt_w1": lat1_w1, "lt_w2": lat1_w2,
                "e1w1": env1_w1, "e1w2": env1_w2}
               for ls, lt, cb in zip(_shard_T(latent_in), _shard_T(lat), cutB)]
    outs = _run(_CACHE["B"], in_maps)
    lw0 = _unshard_T(outs, "lwT", 1280)
    lat2 = _unshard_T(outs, "lat2T", 128)
    w_env1 = _unshard_T(outs, "we1T", 32)

    def contract(lw, s1, s2, v1, v2, v3):
        ws = lw[:, :2 * MUL * MUL].reshape(E, 2, MUL, MUL)
        wv = lw[:, 2 * MUL * MUL:].reshape(E, 3, MUL, MUL)
        S = np.stack([s1, s2], axis=1)
        V = np.stack([v1, v2, v3], axis=1)
        fs = np.einsum('epu,epuv->ev', S, ws, optimize=True) \
            * np.float32(1.0 / np.sqrt(2.0 * MUL))
        fv = np.einsum('epui,epuv->evi', V, wv, optimize=True) \
            * np.float32(1.0 / np.sqrt(3.0 * MUL))
        return fs.astype(np.float32), fv.astype(np.float32)

    fs, fv = contract(lw0, s1, s2, v1, v2, v3)

    # ---- layer 1 host glue
    es, ev = env_block(w_env1, 1)
    s1, s2, v1, v2, v3 = tp(fs, fv, es, ev)
    scalars = np.stack([s1, s2], axis=-1).reshape(E, 2 * MUL)
    latent_in = np.concatenate([lat2, scalars], axis=-1)

    # ---- launch C
    if "C" not in _CACHE:
        _CACHE["C"] = _build_C()
    in_maps = [{"liT": ls,
                "lw_w1": l2w1_w1, "lw_w2": l2w1_w2,
                "f_w1": fl2w_w1, "f_w2": fl2w_w2}
               for ls in _shard_T(latent_in)]
    outs = _run(_CACHE["C"], in_maps)
    lw1 = _unshard_T(outs, "lwT", 1280)
    fw = _unshard_T(outs, "fwT", 16)

    _, fv = contract(lw1, s1, s2, v1, v2, v3)
    return (np.einsum('eu,eui->ei', fw, fv) * inv_sm).astype(np.float32)

